# revision 10
# baseline (speedup 1.0000x reference)
"""CLUB loss kernel for Trainium2 (8 NeuronCores, SPMD row-sharded).

Math: the reference returns mean_i(pos_i - neg_i), a scalar.  Both the
pos and neg terms collapse into sums that never materialize the NxN
distance matrix:

  mean_pos = -0.5/N * (A - 2B + C)
      A = sum_{i,d} x[i,d]^2 * invv[i,d]
      B = sum_{i,d} x[i,d] * mu[i,d] * invv[i,d]
      C = sum_{i,d} mu[i,d]^2 * invv[i,d]
  mean_neg = -0.5 * (S_invv . S_x2 - 2 * S_muinvv . S_x + N*C) / N^2
      S_invv = sum_i invv[i,:]     S_muinvv = sum_i mu[i,:]*invv[i,:]
      S_x    = sum_j x[j,:]        S_x2     = sum_j x[j,:]^2
  loss = mean_pos - mean_neg

Each core handles 2048 rows (2 batches of x + matching mu/logvar rows)
and emits f32 partial sums; the host combines them in float64.

Layout: d-major (128, 1024): partition q = (sub-slab b, dim d), free
axis = row index.  Every reduction is a free-axis row-sum that rides as
an accum_out on the op that produces (or merely streams) the tensor.

v2 structure (from trace analysis of the 23us baseline):
- 6 half-tensor DMAs spread over THREE HWDGE queues (SP, ACT, DVE) so
  the 1.5MB input streams at the ~330GB/s two-queue-plus rate instead
  of one queue's ~190GB/s; triggers are issued before any compute so
  the ~650ns DMA_SEQ time never blocks a compute engine mid-chain.
- 13 compute instructions balanced across ACT (exp, sq, Sx-copy),
  DVE (muinvv, B, A) and Pool (C) - the baseline's 9-instruction ACT
  chain (with 9 x 278ns accumulator reads) was the compute bottleneck.
- one tile pool, per-engine scratch tiles, no memsets: fewer
  cross-engine sync events shrinks the multi-microsecond semaphore
  teardown tail that counts toward exec time.
"""

import sys

sys.path.insert(0, "/opt/trn_rl_repo")

import numpy as np
from contextlib import ExitStack

import concourse.bass as bass
import concourse.bacc as bacc
import concourse.tile as tile
from concourse import mybir
from concourse.bass_utils import run_bass_kernel_spmd

F32 = mybir.dt.float32
N_CORES = 8
B, D, H, W = 16, 64, 32, 32
HW = H * W                # 1024
N = B * HW                # 16384
NB = B // N_CORES         # 2 sub-slabs (batches) per core
ROWS = NB * HW            # 2048 rows per core
COLS = HW                 # free size of the (128, 1024) layout
HALF = COLS // 2

# accumulator column map: name -> [h0 col, h1 col].  The two sums that
# finish last (C h1, A h1) sit in the final columns so the split output
# DMA ships cols [0,12) early and only cols [12,14) after the last op.
ACC_COLS = {
    "Sinvv": [0, 1],
    "Sx2": [2, 3],
    "Sx": [4, 5],
    "Smuinvv": [6, 7],
    "B": [8, 9],
    "A": [10, 13],
    "C": [11, 12],
}
NACC = 14
NACC1 = 12  # cols [0, NACC1) go in the early output DMA

def build_nc() -> bass.Bass:
    nc = bacc.Bacc()
    ins = {
        nm: nc.dram_tensor(nm, [128, HALF], F32, kind="ExternalInput")
        for nm in ("lv0", "lv1", "mu0", "mu1", "x0", "x1")
    }
    accs = nc.dram_tensor("accs", [128, NACC], F32, kind="ExternalOutput")

    with ExitStack() as ctx:
        tc = ctx.enter_context(tile.TileContext(nc))
        pool = ctx.enter_context(tc.tile_pool(name="p", bufs=1))

        lv = pool.tile([128, COLS], F32)
        mu = pool.tile([128, COLS], F32)
        xb = pool.tile([128, COLS], F32)
        invv = pool.tile([128, COLS], F32)
        muinvv = pool.tile([128, COLS], F32)
        x2 = pool.tile([128, COLS], F32)
        cmul = pool.tile([128, COLS], F32)  # mu * muinvv (Pool product)
        gA = pool.tile([128, COLS], F32)    # ACT scratch
        gD = pool.tile([128, COLS], F32)    # DVE scratch
        acc = pool.tile([128, NACC], F32)

        HS = [slice(0, HALF), slice(HALF, COLS)]

        def col(q, h):
            c = ACC_COLS[q][h]
            return acc[:, c:c + 1]

        # Strict-order pins: each emitted op gets a min-sim-time 50us
        # after the previous one, so the tile scheduler cannot reorder
        # anything (its own DMA-latency model mispredicts SWDGE vs HWDGE
        # arrival order and otherwise causes head-of-line blocking on
        # the in-order engine queues).  Sim-time only - no HW waits.
        pin_t = [0.0]

        def pin():
            pin_t[0] += 50.0
            tc.tile_set_cur_wait(pin_t[0] / 1000.0)

        EXP = mybir.ActivationFunctionType.Exp
        SQ = mybir.ActivationFunctionType.Square
        CP = mybir.ActivationFunctionType.Copy
        M = mybir.AluOpType.mult
        X = mybir.AxisListType.X
        ADD = mybir.AluOpType.add

        def act(fn, out, in_, q, h, scale=1.0):
            nc.scalar.activation(
                out=out, in_=in_, func=fn, bias=0.0, scale=scale,
                accum_out=col(q, h),
            )

        def stt(out, in0, in1, q, h):
            nc.vector.scalar_tensor_tensor(
                out=out, in0=in0, scalar=1.0, in1=in1, op0=M, op1=M,
                accum_out=col(q, h),
            )

        # DMA triggers first.  Queue loads balanced against measured
        # queue start times (SP ~8.2us, ACT-q ~9.6, SWDGE ~10.3):
        # SP carries lv0, lv1, x1; ACT-q carries mu0, mu1; SWDGE x0.
        pin()
        nc.sync.dma_start(out=lv[:, HS[0]], in_=ins["lv0"][:, :])
        pin()
        nc.sync.dma_start(out=lv[:, HS[1]], in_=ins["lv1"][:, :])
        pin()
        nc.sync.dma_start(out=xb[:, HS[1]], in_=ins["x1"][:, :])
        pin()
        nc.scalar.dma_start(out=mu[:, HS[0]], in_=ins["mu0"][:, :])
        pin()
        nc.scalar.dma_start(out=mu[:, HS[1]], in_=ins["mu1"][:, :])
        pin()
        nc.gpsimd.dma_start(out=xb[:, HS[0]], in_=ins["x0"][:, :])

        # Compute, emitted in global pin order (producers before
        # consumers so the dep tracker sees every edge).
        pin()
        act(EXP, invv[:, HS[0]], lv[:, HS[0]], "Sinvv", 0, scale=-1.0)
        pin()
        act(EXP, invv[:, HS[1]], lv[:, HS[1]], "Sinvv", 1, scale=-1.0)
        pin()
        stt(muinvv[:, HS[0]], mu[:, HS[0]], invv[:, HS[0]], "Smuinvv", 0)
        pin()
        nc.gpsimd.tensor_mul(cmul[:, HS[0]], mu[:, HS[0]], muinvv[:, HS[0]])
        pin()
        stt(gD[:, HS[0]], xb[:, HS[0]], muinvv[:, HS[0]], "B", 0)
        pin()
        act(SQ, x2[:, HS[0]], xb[:, HS[0]], "Sx2", 0)
        pin()
        stt(muinvv[:, HS[1]], mu[:, HS[1]], invv[:, HS[1]], "Smuinvv", 1)
        pin()
        nc.gpsimd.tensor_mul(cmul[:, HS[1]], mu[:, HS[1]], muinvv[:, HS[1]])
        pin()
        act(SQ, x2[:, HS[1]], xb[:, HS[1]], "Sx2", 1)
        pin()
        stt(gD[:, HS[0]], x2[:, HS[0]], invv[:, HS[0]], "A", 0)
        pin()
        stt(gD[:, HS[1]], xb[:, HS[1]], muinvv[:, HS[1]], "B", 1)
        pin()
        act(CP, gA[:, HS[0]], xb[:, HS[0]], "Sx", 0)
        pin()
        nc.vector.tensor_reduce(
            out=col("Sx", 1), in_=xb[:, HS[1]], axis=X, op=ADD)
        pin()
        act(CP, gA[:, HS[1]], cmul[:, HS[0]], "C", 0)
        pin()
        nc.vector.tensor_reduce(
            out=col("C", 1), in_=cmul[:, HS[1]], axis=X, op=ADD)
        pin()
        # early output DMA: cols [0,12) are complete before A1 lands
        nc.sync.dma_start(out=accs[:, 0:NACC1], in_=acc[:, 0:NACC1])
        pin()
        stt(gD[:, HS[1]], x2[:, HS[1]], invv[:, HS[1]], "A", 1)
        pin()
        nc.sync.dma_start(out=accs[:, NACC1:NACC], in_=acc[:, NACC1:NACC])
    return nc


def _ensure_ntff_hook():
    """This image's antenv lacks axon_hooks; if tracing is requested
    (e.g. BASS_TRACE=1), run_bass_kernel_spmd would die on the import.
    Register the ctypes-based hook if available, else a None hook so
    tracing degrades gracefully."""
    import types

    if "antenv.axon_hooks" in sys.modules:
        return
    try:
        import antenv.axon_hooks  # noqa: F401
        return
    except ImportError:
        pass
    hook = None
    try:
        sys.path.insert(0, "/root/.axon_site")
        from trn_agent_boot.trn_boot import _ntff_profile_via_ctypes

        hook = _ntff_profile_via_ctypes("/opt/axon/libaxon_pjrt.so")
    except Exception:
        hook = None
    mod = types.ModuleType("antenv.axon_hooks")
    mod._hook = hook
    mod.get_axon_ntff_profile_hook = lambda: mod._hook
    mod.set_axon_ntff_profile_hook = lambda h: setattr(mod, "_hook", h)
    sys.modules["antenv.axon_hooks"] = mod


_ensure_ntff_hook()

_NC = None


def _get_nc():
    global _NC
    if _NC is None:
        _NC = build_nc()
        # bacc passes legalize multi-sync-wait instructions for TRN2 codegen
        _NC.compile()
    return _NC


def make_in_maps(x, mu, logvar):
    x = np.ascontiguousarray(np.asarray(x, dtype=np.float32))
    mu = np.asarray(mu, dtype=np.float32)
    lv = np.asarray(logvar, dtype=np.float32)
    in_maps = []
    for c in range(N_CORES):
        r0 = c * ROWS
        mu_t = np.concatenate(
            [mu[r0 + b * HW:r0 + (b + 1) * HW].T for b in range(NB)], axis=0
        )
        lv_t = np.concatenate(
            [lv[r0 + b * HW:r0 + (b + 1) * HW].T for b in range(NB)], axis=0
        )
        x_t = x[c * NB:(c + 1) * NB].reshape(128, COLS)
        m = {}
        for h, sl in enumerate((slice(0, HALF), slice(HALF, COLS))):
            m[f"lv{h}"] = np.ascontiguousarray(lv_t[:, sl])
            m[f"mu{h}"] = np.ascontiguousarray(mu_t[:, sl])
            m[f"x{h}"] = np.ascontiguousarray(x_t[:, sl])
        in_maps.append(m)
    return in_maps


def combine(results) -> np.ndarray:
    tot = {q: np.zeros(128, dtype=np.float64) for q in ACC_COLS}
    for r in results:
        a = np.asarray(r["accs"], dtype=np.float64)  # (128, NACC)
        for q, cols in ACC_COLS.items():
            tot[q] += a[:, cols].sum(axis=1)
    A, Bs, C = (tot[q].sum() for q in ("A", "B", "C"))
    vec = {q: tot[q].reshape(NB, D).sum(axis=0)
           for q in ("Sx", "Sx2", "Sinvv", "Smuinvv")}
    mean_pos = -0.5 / N * (A - 2.0 * Bs + C)
    mean_D = (vec["Sinvv"] @ vec["Sx2"] - 2.0 * vec["Smuinvv"] @ vec["Sx"]
              + N * C) / float(N) ** 2
    loss = mean_pos + 0.5 * mean_D
    return np.array(loss, dtype=np.float32)


def kernel(x, mu, logvar, **_kwargs):
    nc = _get_nc()
    in_maps = make_in_maps(x, mu, logvar)
    res = run_bass_kernel_spmd(nc, in_maps, list(range(N_CORES)))
    return combine(res.results)
